# revision 25
# baseline (speedup 1.0000x reference)
"""Trainium2 Bass kernel for nn_NeuralNetwork_31447750541324.

Network: per-frame conv stack (stride==kernel convs -> pure matmuls) ->
BatchNorm1d over (B, len) -> per-sample channel reorder by range ->
3 Elman RNNs (input 1, hidden 256) over F=64 steps -> mean -> linear.

Two launches:
  A) conv1 (9x9/9 as matmul, K=243 zero-padded to 2x128 chunks: partial-
     partition DMAs fan unevenly over the 16 DMA engines and halve
     bandwidth) + relu + 2x2 maxpool on all 8 cores, data-parallel over
     the 640 frames.  All frame DMAs are issued upfront on the sync
     queue (~307 GB/s/core cap); the first group goes in 2-frame pieces
     to fill the queue's ~2.7us cold-start window and the last group is
     split so its pairs start early.  pool1 [64ch, 10, 10] f16 per frame
     is DMA'd back out per-group (per-pair for the last group).  conv2
     (12 MFLOP) + pool2 + BN + range-sort run on the host between
     launches.
  B) the 3 RNNs on 3 cores (one each).  All 64 input injections are
     matmul'd upfront into a fixed psum layout [128, seg, mc, 512] (one
     disjoint [2,10] slice per step, no psum recycling), then the loop
     is only 4 recurrent matmuls + tanh per step - the measured
     structural floor (~686ns: 277 tanh pipeline + 250 matmul
     issue/drain + ~160 semaphore hops).  The final hidden state is
     DMA'd out; the mean+linear head runs on the host.

fp8 was evaluated and rejected: e4m3 conv1 gives 18% rel err and flips
the range-sort permutation; e4m3 recurrence gives 4.1e-2 (gate: 2e-2).
Cross-core pipelining of the RNN under the conv stream was evaluated
and rejected: collectives cost 6.7us steady-state with a ~55us
first-use penalty that serializes ahead of the RNN chain.
"""

import os
import numpy as np

# ---------------- static problem dims ----------------
B, F, C, H, W = 10, 64, 3, 180, 180
NF = B * F                      # 640 frames
NCORES = 8
FPC = NF // NCORES              # 80 frames per core
CH, OUT, NCLS = 64, 256, 5
K1, K2 = 9, 5
KC1 = C * K1 * K1               # 243 contraction rows
KB = 115                        # chunk-1 rows (243 - 128)
N1 = 400                        # 20x20 conv1 output positions
EPS = 1e-5

GRP = 8                         # frames per DMA group
NGRP = FPC // GRP               # 10 groups per core
NPAIR = FPC // 2                # 40 psum pairs per core

_cache = {}


# ---------------- launch A: conv1+pool1, 8 cores ----------------
def _build_conv_nc():
    import concourse.bacc as bacc
    import concourse.bass as bass
    import concourse.mybir as mybir
    import concourse.tile as tile

    f16, f32 = mybir.dt.float16, mybir.dt.float32
    nc = bacc.Bacc("TRN2", target_bir_lowering=False, debug=False,
                   num_devices=NCORES)

    pA = nc.dram_tensor("pA", [NGRP, 128, GRP, N1], f16, kind="ExternalInput")
    pB = nc.dram_tensor("pB", [NGRP, 128, GRP, N1], f16, kind="ExternalInput")
    w1 = nc.dram_tensor("w1", [128, 2, 128], f16, kind="ExternalInput")
    b1 = nc.dram_tensor("b1", [128, 1], f32, kind="ExternalInput")
    po = nc.dram_tensor("po", [128, NPAIR, 100], f16, kind="ExternalOutput")

    Relu = mybir.ActivationFunctionType.Relu
    X = mybir.AxisListType.X
    mx = mybir.AluOpType.max

    with tile.TileContext(nc) as tc:
        with (
            tc.tile_pool(name="const", bufs=1) as cp,
            tc.tile_pool(name="fa", bufs=NGRP) as fpa,
            tc.tile_pool(name="fb", bufs=NGRP) as fpb,
            tc.tile_pool(name="red", bufs=4) as rp,
            tc.tile_pool(name="ps1", bufs=4, space=bass.MemorySpace.PSUM) as pp1,
        ):
            w1s = cp.tile([128, 2, 128], f16, tag="w1")
            b1s = cp.tile([128, 1], f32, tag="b1")
            pool1 = cp.tile([128, NPAIR, 100], f16, tag="pool1")
            nc.scalar.dma_start(w1s[:], w1[:])
            nc.scalar.dma_start(b1s[:], b1[:])

            # issue every frame DMA upfront on the sync queue, whose DGE
            # fans packets evenly across all 16 DMA engines
            gta, gtb = [], []
            for g in range(NGRP):
                ta = fpa.tile([128, GRP, N1], f16, tag="fa")
                tb = fpb.tile([128, GRP, N1], f16, tag="fb")
                if g == 0:
                    # small leading transfers so packets flow during the
                    # queue's cold-start latency
                    for q in range(0, GRP, 2):
                        nc.sync.dma_start(ta[:, q:q + 2], pA[g, :, q:q + 2])
                        nc.sync.dma_start(tb[:, q:q + 2], pB[g, :, q:q + 2])
                elif g < NGRP - 1:
                    nc.sync.dma_start(ta[:], pA[g])
                    nc.sync.dma_start(tb[:], pB[g])
                else:
                    # split the last group so its first pairs can start
                    # while the second half is still in flight
                    for q in range(0, GRP, 2):
                        nc.sync.dma_start(ta[:, q:q + 2], pA[g, :, q:q + 2])
                        nc.sync.dma_start(tb[:, q:q + 2], pB[g, :, q:q + 2])
                gta.append(ta)
                gtb.append(tb)

            for g in range(NGRP):
                ta, tb = gta[g], gtb[g]
                for p in range(GRP // 2):
                    fa, fb = 2 * p, 2 * p + 1
                    pr = g * (GRP // 2) + p
                    ps = pp1.tile([128, 100, 4], f32, tag="ps")
                    nc.tensor.matmul(ps[0:64], w1s[:, 0, 0:64],
                                     ta[:, fa, :], start=True, stop=False)
                    nc.tensor.matmul(ps[64:128], w1s[:, 0, 64:128],
                                     ta[:, fb, :], start=True, stop=False)
                    nc.tensor.matmul(ps[0:64], w1s[:, 1, 0:64],
                                     tb[:, fa, :], start=False, stop=True)
                    nc.tensor.matmul(ps[64:128], w1s[:, 1, 64:128],
                                     tb[:, fb, :], start=False, stop=True)
                    rt = rp.tile([128, 100], f32, tag="rt")
                    nc.vector.tensor_reduce(rt[:], ps[:], axis=X, op=mx)
                    nc.scalar.activation(pool1[:, pr, :], rt[:],
                                         Relu, bias=b1s[:])
                # stream finished pairs out on the scalar queue; the last
                # group goes per-pair so the final write isn't gated on all
                # four activations
                if g < NGRP - 1:
                    sl = slice(g * (GRP // 2), (g + 1) * (GRP // 2))
                    nc.scalar.dma_start(po[:, sl, :], pool1[:, sl, :])
                else:
                    for p in range(GRP // 2):
                        pr_ = g * (GRP // 2) + p
                        nc.scalar.dma_start(po[:, pr_:pr_ + 1, :],
                                            pool1[:, pr_:pr_ + 1, :])

    nc.compile()
    return nc


# ---------------- launch B: one RNN per core, 3 cores ----------------
NSLOT = 4                       # psum rotation depth


def _build_rnn_nc():
    import concourse.bacc as bacc
    import concourse.bass as bass
    import concourse.mybir as mybir
    import concourse.tile as tile

    f16, f32 = mybir.dt.float16, mybir.dt.float32
    nc = bacc.Bacc("TRN2", target_bir_lowering=False, debug=False,
                   num_devices=3)

    # one packed input tensor: [128, 640 + 512 + 256 + 10] f16
    #   xb  [128, 64, 10]  (row0 = x_t, row1 = 1.0)
    #   wh  [128, 2, 2, 128] recurrent weights (kc, mc chunks)
    #   cf  [128, 2, 128]  injection weights (row0 = Wih, row1 = bih+bhh)
    #   wl  [128, 2, 5]    final linear (Wl.T / 3)
    NX, NW, NC_, NL = F * B, 512, 256, 10
    blob = nc.dram_tensor("blob", [128, NX + NW + NC_ + NL], f16,
                          kind="ExternalInput")
    ph = nc.dram_tensor("ph", [128, 2, B], f16, kind="ExternalOutput")

    Tanh = mybir.ActivationFunctionType.Tanh

    with tile.TileContext(nc) as tc:
        with (
            tc.tile_pool(name="const", bufs=1) as cp,
            tc.tile_pool(name="h", bufs=3) as hp,
            tc.tile_pool(name="ps", bufs=1, space=bass.MemorySpace.PSUM) as pp,
        ):
            blos = cp.tile([128, NX + NW + NC_ + NL], f16, tag="blob")
            # xb+cf arrive first so the injection matmuls can start while
            # the recurrent weights are still in flight
            HX = NC_ + 500
            nc.sync.dma_start(blos[:, 0:HX], blob[:, 0:HX])
            nc.gpsimd.dma_start(blos[:, HX:NC_ + NX], blob[:, HX:NC_ + NX])
            nc.scalar.dma_start(blos[:, NC_ + NX:], blob[:, NC_ + NX:])
            cfs = blos[:, 0:NC_].rearrange("p (mc m) -> p mc m", mc=2)
            xbs = blos[:, NC_:NC_ + NX].rearrange("p (t b) -> p t b", t=F)
            whs = blos[:, NC_ + NX:NC_ + NX + NW].rearrange(
                "p (kc mc m) -> p kc mc m", kc=2, mc=2)

            # One psum region [128, seg, mc, 512] f32 = 4 banks.  Step t
            # lives at (seg = t // 50, offset = 10*(t % 50)); every step
            # has its own disjoint [2, 10] slice, so the loop has no psum
            # recycling (no WAR against the tanh reads).  All 64 injection
            # matmuls run upfront (one per seg x mc region, K=2), and the
            # per-step recurrent matmuls accumulate on top.
            SEG = 50
            pt = pp.tile([128, 2, 2, 512], f32, tag="pt")

            for seg in range(2):
                t0, t1 = seg * SEG, min((seg + 1) * SEG, F)
                n = (t1 - t0) * B
                for mc in range(2):
                    nc.tensor.matmul(
                        pt[:, seg, mc, 0:n],
                        cfs[0:2, mc, :],
                        xbs[0:2, t0:t1, :], start=True, stop=True)

            def pslot(t):
                seg, off = t // SEG, B * (t % SEG)
                return pt[:, seg, :, off:off + B], pt[:, seg, 0, off:off + B], \
                    pt[:, seg, 1, off:off + B]

            h = None
            for t in range(F):
                both, p0, p1 = pslot(t)
                if t > 0:
                    for mc, px in ((0, p0), (1, p1)):
                        nc.tensor.matmul(px, whs[:, 0, mc, :], h[:, 0, :],
                                         start=False, stop=False,
                                         skip_group_check=True)
                        nc.tensor.matmul(px, whs[:, 1, mc, :], h[:, 1, :],
                                         start=False, stop=True,
                                         skip_group_check=True)
                ht = hp.tile([128, 2, B], f16, tag="h")
                nc.scalar.activation(ht[:], both, Tanh)
                h = ht

            nc.sync.dma_start(ph[:], h[:])

    nc.compile()
    return nc


# ---------------- host-side prep + glue ----------------
def _prep_conv_inputs(x, W1, b1):
    # im2col for conv1: stride==kernel => non-overlapping patches.
    # k-order (c, kh, kw); n-order (oh10, ow10, ph, pw) so the last free
    # axis of the psum groups each 2x2 maxpool window.
    xv = x.reshape(NF, C, 10, 2, 9, 10, 2, 9)
    pat = xv.transpose(0, 1, 4, 7, 2, 5, 3, 6).reshape(NF, KC1, N1)
    pat = pat.astype(np.float16)
    patp = np.zeros((NF, 256, N1), np.float16)
    patp[:, 0:KC1] = pat
    patp = patp.reshape(NCORES, NGRP, GRP, 2, 128, N1)
    pA = np.ascontiguousarray(patp[:, :, :, 0].transpose(0, 1, 3, 2, 4))
    pB = np.ascontiguousarray(patp[:, :, :, 1].transpose(0, 1, 3, 2, 4))

    w1m = np.zeros((2 * 128, 64), np.float16)
    w1m[:KC1] = W1.reshape(64, KC1).T               # [K, M]
    w1c = w1m.reshape(2, 128, 64).transpose(1, 0, 2)
    w1t = np.ascontiguousarray(np.concatenate([w1c, w1c], axis=2))

    b1d = np.concatenate([b1, b1]).reshape(128, 1).astype(np.float32)
    return pA, pB, w1t, b1d


def _host_conv2_bn(po_list, W2, b2, gamma, beta):
    # reassemble pool1 [NF, 64, 10, 10] from per-core [128, NPAIR, 100]
    pool1 = np.empty((NF, 64, 100), np.float32)
    for k, r in enumerate(po_list):
        v = np.asarray(r, np.float32)               # [128, 40, 100]
        base = k * FPC
        idx = base + 2 * np.arange(NPAIR)
        pool1[idx] = v[0:64].transpose(1, 0, 2)
        pool1[idx + 1] = v[64:128].transpose(1, 0, 2)
    pool1 = pool1.reshape(NF, 64, 10, 10)

    # conv2 5x5 stride 5 + relu + 2x2 maxpool -> y [B, F, 3]
    w2m = W2.reshape(3, 64 * 25)
    yv = pool1.reshape(NF, 64, 2, 5, 2, 5).transpose(0, 1, 3, 5, 2, 4)
    yv = np.ascontiguousarray(yv).reshape(NF, 64 * 25, 4)
    z = np.einsum('fkn,mk->fmn', yv, w2m, optimize=True) \
        + b2[None, :, None]
    z = np.maximum(z, 0.0).max(axis=2)              # [NF, 3]
    y = z.reshape(B, F, 3)

    mean = y.mean(axis=(0, 2), keepdims=True)
    var = y.var(axis=(0, 2), keepdims=True)
    yn = (y - mean) / np.sqrt(var + EPS) * gamma[None, :, None] \
        + beta[None, :, None]
    return yn


def _prep_rnn_blob(ts_r, Wih_r, Whh_r, bih_r, bhh_r, Wl):
    # ts_r: [F, B] f32 rank-r input sequence
    NX = F * B
    blob = np.zeros((128, NX + 512 + 256 + 10), np.float16)
    xb = np.zeros((128, F, B), np.float16)
    xb[0] = ts_r
    xb[1] = 1.0
    cf = np.zeros((128, 2, 128), np.float16)
    bsum = bih_r + bhh_r
    for mc in range(2):
        cf[0, mc, :] = Wih_r[mc * 128:(mc + 1) * 128, 0]
        cf[1, mc, :] = bsum[mc * 128:(mc + 1) * 128]
    blob[:, 0:256] = cf.reshape(128, 256)
    blob[:, 256:256 + NX] = xb.reshape(128, NX)
    wh = np.zeros((128, 2, 2, 128), np.float16)
    WhhT = Whh_r.T
    for kc in range(2):
        for mc in range(2):
            wh[:, kc, mc, :] = WhhT[kc * 128:(kc + 1) * 128,
                                    mc * 128:(mc + 1) * 128]
    blob[:, 256 + NX:256 + NX + 512] = wh.reshape(128, 512)
    wl = np.zeros((128, 2, 5), np.float16)
    WlT3 = (Wl.T / 3.0)
    for kc in range(2):
        wl[:, kc, :] = WlT3[kc * 128:(kc + 1) * 128]
    blob[:, 256 + NX + 512:] = wl.reshape(128, 10)
    return blob


def _ensure_profile_hook():
    """antenv.axon_hooks is absent in this image; synthesize it so
    run_bass_kernel_spmd(trace=True) can capture NTFF profiles."""
    import sys
    import types
    try:
        from antenv.axon_hooks import get_axon_ntff_profile_hook  # noqa
        return True
    except ImportError:
        pass
    try:
        sys.path.insert(0, "/root/.axon_site/trn_agent_boot")
        from trn_boot import _ntff_profile_via_ctypes
        hook = _ntff_profile_via_ctypes("/opt/axon/libaxon_pjrt.so")
        if hook is None:
            return False
        import antenv
        mod = types.ModuleType("antenv.axon_hooks")
        mod._hook = hook
        mod.get_axon_ntff_profile_hook = lambda: mod._hook
        mod.set_axon_ntff_profile_hook = lambda h: setattr(mod, "_hook", h)
        sys.modules["antenv.axon_hooks"] = mod
        antenv.axon_hooks = mod
        return True
    except Exception:
        return False


def _run(nc, in_maps, core_ids, label):
    from concourse.bass_utils import run_bass_kernel_spmd
    trace = os.environ.get("KERNEL_TRACE", "0") == "1"
    if trace:
        trace = _ensure_profile_hook()
    kw = {}
    if trace:
        import tempfile
        tdir = tempfile.mkdtemp(prefix=f"ktrace_{label}_")
        kw = {"tmpdir": tdir}
    res = run_bass_kernel_spmd(nc, in_maps, core_ids, trace=trace, **kw)
    _cache.setdefault("exec_ns", {})[label] = res.exec_time_ns
    _cache.setdefault("results_obj", {})[label] = res
    return res.results


# ---------------- main entry ----------------
def kernel(x, W1, b1, W2, b2, gamma, beta, Wih, Whh, bih, bhh, Wl, bl):
    x, W1, b1, W2, b2 = map(np.asarray, (x, W1, b1, W2, b2))
    gamma, beta = np.asarray(gamma), np.asarray(beta)
    Wih, Whh, bih, bhh = map(np.asarray, (Wih, Whh, bih, bhh))
    Wl, bl = np.asarray(Wl), np.asarray(bl)

    if "conv" not in _cache:
        _cache["conv"] = _build_conv_nc()
    if "rnn" not in _cache:
        _cache["rnn"] = _build_rnn_nc()

    # ---- launch A: conv1 + pool1 over 640 frames on 8 cores ----
    pA, pB, w1t, b1c = _prep_conv_inputs(x.reshape(NF, C, H, W), W1, b1)
    in_maps = [
        {"pA": pA[k], "pB": pB[k], "w1": w1t, "b1": b1c}
        for k in range(NCORES)
    ]
    res = _run(_cache["conv"], in_maps, list(range(NCORES)), "conv")

    # ---- host glue: conv2 + pool2 + BN + per-sample channel reorder ----
    yn = _host_conv2_bn([r["po"] for r in res], W2, b2, gamma, beta)
    t = yn.transpose(0, 2, 1)                        # [B, 3, F]
    rng = t.max(-1) - t.min(-1)
    perm = np.argsort(rng, axis=1, kind="stable")
    tsel = np.take_along_axis(t, perm[:, :, None], axis=1)  # [B, 3, F]

    # ---- launch B: 3 RNNs on 3 cores (+ scaled final linear) ----
    in_maps_b = []
    for r in range(3):
        blob = _prep_rnn_blob(tsel[:, r, :].T, Wih[r], Whh[r],
                              bih[r], bhh[r], Wl)
        in_maps_b.append({"blob": blob})
    res_b = _run(_cache["rnn"], in_maps_b, [0, 1, 2], "rnn")

    avg = np.zeros((B, OUT), np.float32)
    for r in range(3):
        hv = np.asarray(res_b[r]["ph"], np.float32)      # [128, 2, B]
        hr = hv.transpose(1, 0, 2).reshape(OUT, B)       # [256, B]
        avg += hr.T / 3.0
    out = avg @ Wl.T + bl[None, :]
    return out.astype(np.float32)


# revision 26
# speedup vs baseline: 1.0439x; 1.0439x over previous
"""Trainium2 Bass kernel for nn_NeuralNetwork_31447750541324.

Network: per-frame conv stack (stride==kernel convs -> pure matmuls) ->
BatchNorm1d over (B, len) -> per-sample channel reorder by range ->
3 Elman RNNs (input 1, hidden 256) over F=64 steps -> mean -> linear.

Two launches:
  A) conv1 (9x9/9 as matmul, K=243 zero-padded to 2x128 chunks: partial-
     partition DMAs fan unevenly over the 16 DMA engines and halve
     bandwidth) + relu + 2x2 maxpool on all 8 cores, data-parallel over
     the 640 frames.  All frame DMAs are issued upfront on the sync
     queue (~307 GB/s/core cap); the first group goes in 2-frame pieces
     to fill the queue's ~2.7us cold-start window and the last group is
     split so its pairs start early.  pool1 [64ch, 10, 10] f16 per frame
     is DMA'd back out per-group (per-pair for the last group).  conv2
     (12 MFLOP) + pool2 + BN + range-sort run on the host between
     launches.
  B) the 3 RNNs on 3 cores (one each).  All 64 input injections are
     matmul'd upfront into a fixed psum layout [128, seg, mc, 512] (one
     disjoint [2,10] slice per step, no psum recycling), then the loop
     is only 4 recurrent matmuls + tanh per step - the measured
     structural floor (~686ns: 277 tanh pipeline + 250 matmul
     issue/drain + ~160 semaphore hops).  The final hidden state is
     DMA'd out; the mean+linear head runs on the host.

fp8 was evaluated and rejected: e4m3 conv1 gives 18% rel err and flips
the range-sort permutation; e4m3 recurrence gives 4.1e-2 (gate: 2e-2).
Cross-core pipelining of the RNN under the conv stream was evaluated
and rejected: collectives cost 6.7us steady-state with a ~55us
first-use penalty that serializes ahead of the RNN chain.
"""

import os
import numpy as np

# ---------------- static problem dims ----------------
B, F, C, H, W = 10, 64, 3, 180, 180
NF = B * F                      # 640 frames
NCORES = 8
FPC = NF // NCORES              # 80 frames per core
CH, OUT, NCLS = 64, 256, 5
K1, K2 = 9, 5
KC1 = C * K1 * K1               # 243 contraction rows
KB = 115                        # chunk-1 rows (243 - 128)
N1 = 400                        # 20x20 conv1 output positions
EPS = 1e-5

GRP = 8                         # frames per DMA group
NGRP = FPC // GRP               # 10 groups per core
NPAIR = FPC // 2                # 40 psum pairs per core

_cache = {}


# ---------------- launch A: conv1+pool1, 8 cores ----------------
def _build_conv_nc():
    import concourse.bacc as bacc
    import concourse.bass as bass
    import concourse.mybir as mybir
    import concourse.tile as tile

    f16, f32 = mybir.dt.float16, mybir.dt.float32
    nc = bacc.Bacc("TRN2", target_bir_lowering=False, debug=False,
                   num_devices=NCORES)

    pA = nc.dram_tensor("pA", [NGRP, 128, GRP, N1], f16, kind="ExternalInput")
    pB = nc.dram_tensor("pB", [NGRP, 128, GRP, N1], f16, kind="ExternalInput")
    w1 = nc.dram_tensor("w1", [128, 2, 128], f16, kind="ExternalInput")
    b1 = nc.dram_tensor("b1", [128, 1], f32, kind="ExternalInput")
    po = nc.dram_tensor("po", [128, NPAIR, 100], f16, kind="ExternalOutput")

    Relu = mybir.ActivationFunctionType.Relu
    X = mybir.AxisListType.X
    mx = mybir.AluOpType.max

    with tile.TileContext(nc) as tc:
        with (
            tc.tile_pool(name="const", bufs=1) as cp,
            tc.tile_pool(name="fa", bufs=NGRP) as fpa,
            tc.tile_pool(name="fb", bufs=NGRP) as fpb,
            tc.tile_pool(name="red", bufs=4) as rp,
            tc.tile_pool(name="ps1", bufs=4, space=bass.MemorySpace.PSUM) as pp1,
        ):
            w1s = cp.tile([128, 2, 128], f16, tag="w1")
            b1s = cp.tile([128, 1], f32, tag="b1")
            pool1 = cp.tile([128, NPAIR, 100], f16, tag="pool1")
            nc.scalar.dma_start(w1s[:], w1[:])
            nc.scalar.dma_start(b1s[:], b1[:])

            # issue every frame DMA upfront on the sync queue, whose DGE
            # fans packets evenly across all 16 DMA engines
            gta, gtb = [], []
            for g in range(NGRP):
                ta = fpa.tile([128, GRP, N1], f16, tag="fa")
                tb = fpb.tile([128, GRP, N1], f16, tag="fb")
                if g == 0:
                    # small leading transfers so packets flow during the
                    # queue's cold-start latency
                    for q in range(0, GRP, 2):
                        nc.sync.dma_start(ta[:, q:q + 2], pA[g, :, q:q + 2])
                        nc.sync.dma_start(tb[:, q:q + 2], pB[g, :, q:q + 2])
                elif g < NGRP - 1:
                    nc.sync.dma_start(ta[:], pA[g])
                    nc.sync.dma_start(tb[:], pB[g])
                else:
                    # split the last group so its first pairs can start
                    # while the second half is still in flight
                    hh = GRP // 2
                    nc.sync.dma_start(ta[:, 0:hh], pA[g, :, 0:hh])
                    nc.sync.dma_start(tb[:, 0:hh], pB[g, :, 0:hh])
                    nc.sync.dma_start(ta[:, hh:], pA[g, :, hh:])
                    nc.sync.dma_start(tb[:, hh:], pB[g, :, hh:])
                gta.append(ta)
                gtb.append(tb)

            for g in range(NGRP):
                ta, tb = gta[g], gtb[g]
                for p in range(GRP // 2):
                    fa, fb = 2 * p, 2 * p + 1
                    pr = g * (GRP // 2) + p
                    ps = pp1.tile([128, 100, 4], f32, tag="ps")
                    nc.tensor.matmul(ps[0:64], w1s[:, 0, 0:64],
                                     ta[:, fa, :], start=True, stop=False)
                    nc.tensor.matmul(ps[64:128], w1s[:, 0, 64:128],
                                     ta[:, fb, :], start=True, stop=False)
                    nc.tensor.matmul(ps[0:64], w1s[:, 1, 0:64],
                                     tb[:, fa, :], start=False, stop=True)
                    nc.tensor.matmul(ps[64:128], w1s[:, 1, 64:128],
                                     tb[:, fb, :], start=False, stop=True)
                    rt = rp.tile([128, 100], f32, tag="rt")
                    nc.vector.tensor_reduce(rt[:], ps[:], axis=X, op=mx)
                    nc.scalar.activation(pool1[:, pr, :], rt[:],
                                         Relu, bias=b1s[:])
                # stream finished pairs out on the scalar queue; the last
                # group goes per-pair so the final write isn't gated on all
                # four activations
                if g < NGRP - 1:
                    sl = slice(g * (GRP // 2), (g + 1) * (GRP // 2))
                    nc.scalar.dma_start(po[:, sl, :], pool1[:, sl, :])
                else:
                    for p in range(GRP // 2):
                        pr_ = g * (GRP // 2) + p
                        nc.scalar.dma_start(po[:, pr_:pr_ + 1, :],
                                            pool1[:, pr_:pr_ + 1, :])

    nc.compile()
    return nc


# ---------------- launch B: one RNN per core, 3 cores ----------------
NSLOT = 4                       # psum rotation depth


def _build_rnn_nc():
    import concourse.bacc as bacc
    import concourse.bass as bass
    import concourse.mybir as mybir
    import concourse.tile as tile

    f16, f32 = mybir.dt.float16, mybir.dt.float32
    nc = bacc.Bacc("TRN2", target_bir_lowering=False, debug=False,
                   num_devices=3)

    # one packed input tensor: [128, 640 + 512 + 256 + 10] f16
    #   xb  [128, 64, 10]  (row0 = x_t, row1 = 1.0)
    #   wh  [128, 2, 2, 128] recurrent weights (kc, mc chunks)
    #   cf  [128, 2, 128]  injection weights (row0 = Wih, row1 = bih+bhh)
    #   wl  [128, 2, 5]    final linear (Wl.T / 3)
    NX, NW, NC_, NL = F * B, 512, 256, 10
    blob = nc.dram_tensor("blob", [128, NX + NW + NC_ + NL], f16,
                          kind="ExternalInput")
    ph = nc.dram_tensor("ph", [128, 2, B], f16, kind="ExternalOutput")

    Tanh = mybir.ActivationFunctionType.Tanh

    with tile.TileContext(nc) as tc:
        with (
            tc.tile_pool(name="const", bufs=1) as cp,
            tc.tile_pool(name="h", bufs=3) as hp,
            tc.tile_pool(name="ps", bufs=1, space=bass.MemorySpace.PSUM) as pp,
        ):
            blos = cp.tile([128, NX + NW + NC_ + NL], f16, tag="blob")
            # xb+cf arrive first so the injection matmuls can start while
            # the recurrent weights are still in flight
            HX = NC_ + 500
            nc.sync.dma_start(blos[:, 0:HX], blob[:, 0:HX])
            nc.gpsimd.dma_start(blos[:, HX:NC_ + NX], blob[:, HX:NC_ + NX])
            nc.scalar.dma_start(blos[:, NC_ + NX:], blob[:, NC_ + NX:])
            cfs = blos[:, 0:NC_].rearrange("p (mc m) -> p mc m", mc=2)
            xbs = blos[:, NC_:NC_ + NX].rearrange("p (t b) -> p t b", t=F)
            whs = blos[:, NC_ + NX:NC_ + NX + NW].rearrange(
                "p (kc mc m) -> p kc mc m", kc=2, mc=2)

            # One psum region [128, seg, mc, 512] f32 = 4 banks.  Step t
            # lives at (seg = t // 50, offset = 10*(t % 50)); every step
            # has its own disjoint [2, 10] slice, so the loop has no psum
            # recycling (no WAR against the tanh reads).  All 64 injection
            # matmuls run upfront (one per seg x mc region, K=2), and the
            # per-step recurrent matmuls accumulate on top.
            SEG = 50
            pt = pp.tile([128, 2, 2, 512], f32, tag="pt")

            for seg in range(2):
                t0, t1 = seg * SEG, min((seg + 1) * SEG, F)
                n = (t1 - t0) * B
                for mc in range(2):
                    nc.tensor.matmul(
                        pt[:, seg, mc, 0:n],
                        cfs[0:2, mc, :],
                        xbs[0:2, t0:t1, :], start=True, stop=True)

            def pslot(t):
                seg, off = t // SEG, B * (t % SEG)
                return pt[:, seg, :, off:off + B], pt[:, seg, 0, off:off + B], \
                    pt[:, seg, 1, off:off + B]

            h = None
            for t in range(F):
                both, p0, p1 = pslot(t)
                if t > 0:
                    for mc, px in ((0, p0), (1, p1)):
                        nc.tensor.matmul(px, whs[:, 0, mc, :], h[:, 0, :],
                                         start=False, stop=False,
                                         skip_group_check=True)
                        nc.tensor.matmul(px, whs[:, 1, mc, :], h[:, 1, :],
                                         start=False, stop=True,
                                         skip_group_check=True)
                ht = hp.tile([128, 2, B], f16, tag="h")
                nc.scalar.activation(ht[:], both, Tanh)
                h = ht

            nc.sync.dma_start(ph[:], h[:])

    nc.compile()
    return nc


# ---------------- host-side prep + glue ----------------
def _prep_conv_inputs(x, W1, b1):
    # im2col for conv1: stride==kernel => non-overlapping patches.
    # k-order (c, kh, kw); n-order (oh10, ow10, ph, pw) so the last free
    # axis of the psum groups each 2x2 maxpool window.
    xv = x.reshape(NF, C, 10, 2, 9, 10, 2, 9)
    pat = xv.transpose(0, 1, 4, 7, 2, 5, 3, 6).reshape(NF, KC1, N1)
    pat = pat.astype(np.float16)
    patp = np.zeros((NF, 256, N1), np.float16)
    patp[:, 0:KC1] = pat
    patp = patp.reshape(NCORES, NGRP, GRP, 2, 128, N1)
    pA = np.ascontiguousarray(patp[:, :, :, 0].transpose(0, 1, 3, 2, 4))
    pB = np.ascontiguousarray(patp[:, :, :, 1].transpose(0, 1, 3, 2, 4))

    w1m = np.zeros((2 * 128, 64), np.float16)
    w1m[:KC1] = W1.reshape(64, KC1).T               # [K, M]
    w1c = w1m.reshape(2, 128, 64).transpose(1, 0, 2)
    w1t = np.ascontiguousarray(np.concatenate([w1c, w1c], axis=2))

    b1d = np.concatenate([b1, b1]).reshape(128, 1).astype(np.float32)
    return pA, pB, w1t, b1d


def _host_conv2_bn(po_list, W2, b2, gamma, beta):
    # reassemble pool1 [NF, 64, 10, 10] from per-core [128, NPAIR, 100]
    pool1 = np.empty((NF, 64, 100), np.float32)
    for k, r in enumerate(po_list):
        v = np.asarray(r, np.float32)               # [128, 40, 100]
        base = k * FPC
        idx = base + 2 * np.arange(NPAIR)
        pool1[idx] = v[0:64].transpose(1, 0, 2)
        pool1[idx + 1] = v[64:128].transpose(1, 0, 2)
    pool1 = pool1.reshape(NF, 64, 10, 10)

    # conv2 5x5 stride 5 + relu + 2x2 maxpool -> y [B, F, 3]
    w2m = W2.reshape(3, 64 * 25)
    yv = pool1.reshape(NF, 64, 2, 5, 2, 5).transpose(0, 1, 3, 5, 2, 4)
    yv = np.ascontiguousarray(yv).reshape(NF, 64 * 25, 4)
    z = np.einsum('fkn,mk->fmn', yv, w2m, optimize=True) \
        + b2[None, :, None]
    z = np.maximum(z, 0.0).max(axis=2)              # [NF, 3]
    y = z.reshape(B, F, 3)

    mean = y.mean(axis=(0, 2), keepdims=True)
    var = y.var(axis=(0, 2), keepdims=True)
    yn = (y - mean) / np.sqrt(var + EPS) * gamma[None, :, None] \
        + beta[None, :, None]
    return yn


def _prep_rnn_blob(ts_r, Wih_r, Whh_r, bih_r, bhh_r, Wl):
    # ts_r: [F, B] f32 rank-r input sequence
    NX = F * B
    blob = np.zeros((128, NX + 512 + 256 + 10), np.float16)
    xb = np.zeros((128, F, B), np.float16)
    xb[0] = ts_r
    xb[1] = 1.0
    cf = np.zeros((128, 2, 128), np.float16)
    bsum = bih_r + bhh_r
    for mc in range(2):
        cf[0, mc, :] = Wih_r[mc * 128:(mc + 1) * 128, 0]
        cf[1, mc, :] = bsum[mc * 128:(mc + 1) * 128]
    blob[:, 0:256] = cf.reshape(128, 256)
    blob[:, 256:256 + NX] = xb.reshape(128, NX)
    wh = np.zeros((128, 2, 2, 128), np.float16)
    WhhT = Whh_r.T
    for kc in range(2):
        for mc in range(2):
            wh[:, kc, mc, :] = WhhT[kc * 128:(kc + 1) * 128,
                                    mc * 128:(mc + 1) * 128]
    blob[:, 256 + NX:256 + NX + 512] = wh.reshape(128, 512)
    wl = np.zeros((128, 2, 5), np.float16)
    WlT3 = (Wl.T / 3.0)
    for kc in range(2):
        wl[:, kc, :] = WlT3[kc * 128:(kc + 1) * 128]
    blob[:, 256 + NX + 512:] = wl.reshape(128, 10)
    return blob


def _ensure_profile_hook():
    """antenv.axon_hooks is absent in this image; synthesize it so
    run_bass_kernel_spmd(trace=True) can capture NTFF profiles."""
    import sys
    import types
    try:
        from antenv.axon_hooks import get_axon_ntff_profile_hook  # noqa
        return True
    except ImportError:
        pass
    try:
        sys.path.insert(0, "/root/.axon_site/trn_agent_boot")
        from trn_boot import _ntff_profile_via_ctypes
        hook = _ntff_profile_via_ctypes("/opt/axon/libaxon_pjrt.so")
        if hook is None:
            return False
        import antenv
        mod = types.ModuleType("antenv.axon_hooks")
        mod._hook = hook
        mod.get_axon_ntff_profile_hook = lambda: mod._hook
        mod.set_axon_ntff_profile_hook = lambda h: setattr(mod, "_hook", h)
        sys.modules["antenv.axon_hooks"] = mod
        antenv.axon_hooks = mod
        return True
    except Exception:
        return False


def _run(nc, in_maps, core_ids, label):
    from concourse.bass_utils import run_bass_kernel_spmd
    trace = os.environ.get("KERNEL_TRACE", "0") == "1"
    if trace:
        trace = _ensure_profile_hook()
    kw = {}
    if trace:
        import tempfile
        tdir = tempfile.mkdtemp(prefix=f"ktrace_{label}_")
        kw = {"tmpdir": tdir}
    res = run_bass_kernel_spmd(nc, in_maps, core_ids, trace=trace, **kw)
    _cache.setdefault("exec_ns", {})[label] = res.exec_time_ns
    _cache.setdefault("results_obj", {})[label] = res
    return res.results


# ---------------- main entry ----------------
def kernel(x, W1, b1, W2, b2, gamma, beta, Wih, Whh, bih, bhh, Wl, bl):
    x, W1, b1, W2, b2 = map(np.asarray, (x, W1, b1, W2, b2))
    gamma, beta = np.asarray(gamma), np.asarray(beta)
    Wih, Whh, bih, bhh = map(np.asarray, (Wih, Whh, bih, bhh))
    Wl, bl = np.asarray(Wl), np.asarray(bl)

    if "conv" not in _cache:
        _cache["conv"] = _build_conv_nc()
    if "rnn" not in _cache:
        _cache["rnn"] = _build_rnn_nc()

    # ---- launch A: conv1 + pool1 over 640 frames on 8 cores ----
    pA, pB, w1t, b1c = _prep_conv_inputs(x.reshape(NF, C, H, W), W1, b1)
    in_maps = [
        {"pA": pA[k], "pB": pB[k], "w1": w1t, "b1": b1c}
        for k in range(NCORES)
    ]
    res = _run(_cache["conv"], in_maps, list(range(NCORES)), "conv")

    # ---- host glue: conv2 + pool2 + BN + per-sample channel reorder ----
    yn = _host_conv2_bn([r["po"] for r in res], W2, b2, gamma, beta)
    t = yn.transpose(0, 2, 1)                        # [B, 3, F]
    rng = t.max(-1) - t.min(-1)
    perm = np.argsort(rng, axis=1, kind="stable")
    tsel = np.take_along_axis(t, perm[:, :, None], axis=1)  # [B, 3, F]

    # ---- launch B: 3 RNNs on 3 cores (+ scaled final linear) ----
    in_maps_b = []
    for r in range(3):
        blob = _prep_rnn_blob(tsel[:, r, :].T, Wih[r], Whh[r],
                              bih[r], bhh[r], Wl)
        in_maps_b.append({"blob": blob})
    res_b = _run(_cache["rnn"], in_maps_b, [0, 1, 2], "rnn")

    avg = np.zeros((B, OUT), np.float32)
    for r in range(3):
        hv = np.asarray(res_b[r]["ph"], np.float32)      # [128, 2, B]
        hr = hv.transpose(1, 0, 2).reshape(OUT, B)       # [256, B]
        avg += hr.T / 3.0
    out = avg @ Wl.T + bl[None, :]
    return out.astype(np.float32)


# revision 27
# speedup vs baseline: 1.0574x; 1.0130x over previous
"""Trainium2 Bass kernel for nn_NeuralNetwork_31447750541324.

Network: per-frame conv stack (stride==kernel convs -> pure matmuls) ->
BatchNorm1d over (B, len) -> per-sample channel reorder by range ->
3 Elman RNNs (input 1, hidden 256) over F=64 steps -> mean -> linear.

Two launches:
  A) conv1 (9x9/9 as matmul, K=243 zero-padded to 2x128 chunks: partial-
     partition DMAs fan unevenly over the 16 DMA engines and halve
     bandwidth) + relu + 2x2 maxpool on all 8 cores, data-parallel over
     the 640 frames.  All frame DMAs are issued upfront on the sync
     queue (~307 GB/s/core cap); the first group goes in 2-frame pieces
     to fill the queue's ~2.7us cold-start window and the last group is
     split so its pairs start early.  pool1 [64ch, 10, 10] f16 per frame
     is DMA'd back out per-group (per-pair for the last group).  conv2
     (12 MFLOP) + pool2 + BN + range-sort run on the host between
     launches.
  B) the 3 RNNs on 3 cores (one each).  All 64 input injections are
     matmul'd upfront into a fixed psum layout [128, seg, mc, 512] (one
     disjoint [2,10] slice per step, no psum recycling), then the loop
     is only 4 recurrent matmuls + tanh per step - the measured
     structural floor (~686ns: 277 tanh pipeline + 250 matmul
     issue/drain + ~160 semaphore hops).  The final hidden state is
     DMA'd out; the mean+linear head runs on the host.

fp8 was evaluated and rejected: e4m3 conv1 gives 18% rel err and flips
the range-sort permutation; e4m3 recurrence gives 4.1e-2 (gate: 2e-2).
Cross-core pipelining of the RNN under the conv stream was evaluated
and rejected: collectives cost 6.7us steady-state with a ~55us
first-use penalty that serializes ahead of the RNN chain.
"""

import os
import numpy as np

# ---------------- static problem dims ----------------
B, F, C, H, W = 10, 64, 3, 180, 180
NF = B * F                      # 640 frames
NCORES = 8
FPC = NF // NCORES              # 80 frames per core
CH, OUT, NCLS = 64, 256, 5
K1, K2 = 9, 5
KC1 = C * K1 * K1               # 243 contraction rows
KB = 115                        # chunk-1 rows (243 - 128)
N1 = 400                        # 20x20 conv1 output positions
EPS = 1e-5

GRP = 8                         # frames per DMA group
NGRP = FPC // GRP               # 10 groups per core
NPAIR = FPC // 2                # 40 psum pairs per core

_cache = {}


# ---------------- launch A: conv1+pool1, 8 cores ----------------
def _build_conv_nc():
    import concourse.bacc as bacc
    import concourse.bass as bass
    import concourse.mybir as mybir
    import concourse.tile as tile

    f16, f32 = mybir.dt.float16, mybir.dt.float32
    nc = bacc.Bacc("TRN2", target_bir_lowering=False, debug=False,
                   num_devices=NCORES)

    pA = nc.dram_tensor("pA", [NGRP, 128, GRP, N1], f16, kind="ExternalInput")
    pB = nc.dram_tensor("pB", [NGRP, 128, GRP, N1], f16, kind="ExternalInput")
    w1 = nc.dram_tensor("w1", [128, 2, 128], f16, kind="ExternalInput")
    b1 = nc.dram_tensor("b1", [128, 1], f32, kind="ExternalInput")
    po = nc.dram_tensor("po", [128, NPAIR, 100], f16, kind="ExternalOutput")

    Relu = mybir.ActivationFunctionType.Relu
    X = mybir.AxisListType.X
    mx = mybir.AluOpType.max

    with tile.TileContext(nc) as tc:
        with (
            tc.tile_pool(name="const", bufs=1) as cp,
            tc.tile_pool(name="fa", bufs=NGRP) as fpa,
            tc.tile_pool(name="fb", bufs=NGRP) as fpb,
            tc.tile_pool(name="red", bufs=4) as rp,
            tc.tile_pool(name="ps1", bufs=4, space=bass.MemorySpace.PSUM) as pp1,
        ):
            w1s = cp.tile([128, 2, 128], f16, tag="w1")
            b1s = cp.tile([128, 1], f32, tag="b1")
            pool1 = cp.tile([128, NPAIR, 100], f16, tag="pool1")
            nc.scalar.dma_start(w1s[:], w1[:])
            nc.scalar.dma_start(b1s[:], b1[:])

            # issue every frame DMA upfront on the sync queue, whose DGE
            # fans packets evenly across all 16 DMA engines
            gta, gtb = [], []
            for g in range(NGRP):
                ta = fpa.tile([128, GRP, N1], f16, tag="fa")
                tb = fpb.tile([128, GRP, N1], f16, tag="fb")
                if g == 0:
                    # small leading transfers so packets flow during the
                    # queue's cold-start latency
                    for q in range(0, GRP, 2):
                        nc.sync.dma_start(ta[:, q:q + 2], pA[g, :, q:q + 2])
                        nc.sync.dma_start(tb[:, q:q + 2], pB[g, :, q:q + 2])
                elif g < NGRP - 1:
                    nc.sync.dma_start(ta[:], pA[g])
                    nc.sync.dma_start(tb[:], pB[g])
                else:
                    # split the last group so its first pairs can start
                    # while the second half is still in flight
                    hh = GRP // 2
                    nc.sync.dma_start(ta[:, 0:hh], pA[g, :, 0:hh])
                    nc.sync.dma_start(tb[:, 0:hh], pB[g, :, 0:hh])
                    nc.sync.dma_start(ta[:, hh:], pA[g, :, hh:])
                    nc.sync.dma_start(tb[:, hh:], pB[g, :, hh:])
                gta.append(ta)
                gtb.append(tb)

            for g in range(NGRP):
                ta, tb = gta[g], gtb[g]
                for p in range(GRP // 2):
                    fa, fb = 2 * p, 2 * p + 1
                    pr = g * (GRP // 2) + p
                    ps = pp1.tile([128, 100, 4], f32, tag="ps")
                    nc.tensor.matmul(ps[0:64], w1s[:, 0, 0:64],
                                     ta[:, fa, :], start=True, stop=False)
                    nc.tensor.matmul(ps[64:128], w1s[:, 0, 64:128],
                                     ta[:, fb, :], start=True, stop=False)
                    nc.tensor.matmul(ps[0:64], w1s[:, 1, 0:64],
                                     tb[:, fa, :], start=False, stop=True)
                    nc.tensor.matmul(ps[64:128], w1s[:, 1, 64:128],
                                     tb[:, fb, :], start=False, stop=True)
                    rt = rp.tile([128, 100], f32, tag="rt")
                    nc.vector.tensor_reduce(rt[:], ps[:], axis=X, op=mx)
                    nc.scalar.activation(pool1[:, pr, :], rt[:],
                                         Relu, bias=b1s[:])
                # stream finished pairs out on the scalar queue; the last
                # group goes per-pair so the final write isn't gated on all
                # four activations
                if g < NGRP - 1:
                    sl = slice(g * (GRP // 2), (g + 1) * (GRP // 2))
                    nc.scalar.dma_start(po[:, sl, :], pool1[:, sl, :])
                else:
                    for p in range(GRP // 2):
                        pr_ = g * (GRP // 2) + p
                        nc.scalar.dma_start(po[:, pr_:pr_ + 1, :],
                                            pool1[:, pr_:pr_ + 1, :])

    nc.compile()
    return nc


# ---------------- launch B: one RNN per core, 3 cores ----------------
NSLOT = 4                       # psum rotation depth


def _build_rnn_nc():
    import concourse.bacc as bacc
    import concourse.bass as bass
    import concourse.mybir as mybir
    import concourse.tile as tile

    f16, f32 = mybir.dt.float16, mybir.dt.float32
    nc = bacc.Bacc("TRN2", target_bir_lowering=False, debug=False,
                   num_devices=3)

    # one packed input tensor: [128, 640 + 512 + 256 + 10] f16
    #   xb  [128, 64, 10]  (row0 = x_t, row1 = 1.0)
    #   wh  [128, 2, 2, 128] recurrent weights (kc, mc chunks)
    #   cf  [128, 2, 128]  injection weights (row0 = Wih, row1 = bih+bhh)
    #   wl  [128, 2, 5]    final linear (Wl.T / 3)
    NX, NW, NC_, NL = F * B, 512, 256, 10
    blob = nc.dram_tensor("blob", [128, NX + NW + NC_ + NL], f16,
                          kind="ExternalInput")
    ph = nc.dram_tensor("ph", [128, 2, B], f16, kind="ExternalOutput")

    Tanh = mybir.ActivationFunctionType.Tanh

    with tile.TileContext(nc) as tc:
        with (
            tc.tile_pool(name="const", bufs=1) as cp,
            tc.tile_pool(name="h", bufs=F) as hp,
            tc.tile_pool(name="ps", bufs=1, space=bass.MemorySpace.PSUM) as pp,
        ):
            blos = cp.tile([128, NX + NW + NC_ + NL], f16, tag="blob")
            # xb+cf arrive first so the injection matmuls can start while
            # the recurrent weights are still in flight
            HX = NC_ + 500
            nc.sync.dma_start(blos[:, 0:HX], blob[:, 0:HX])
            nc.gpsimd.dma_start(blos[:, HX:NC_ + NX], blob[:, HX:NC_ + NX])
            nc.scalar.dma_start(blos[:, NC_ + NX:], blob[:, NC_ + NX:])
            cfs = blos[:, 0:NC_].rearrange("p (mc m) -> p mc m", mc=2)
            xbs = blos[:, NC_:NC_ + NX].rearrange("p (t b) -> p t b", t=F)
            whs = blos[:, NC_ + NX:NC_ + NX + NW].rearrange(
                "p (kc mc m) -> p kc mc m", kc=2, mc=2)

            # One psum region [128, seg, mc, 512] f32 = 4 banks.  Step t
            # lives at (seg = t // 50, offset = 10*(t % 50)); every step
            # has its own disjoint [2, 10] slice, so the loop has no psum
            # recycling (no WAR against the tanh reads).  All 64 injection
            # matmuls run upfront (one per seg x mc region, K=2), and the
            # per-step recurrent matmuls accumulate on top.
            SEG = 50
            pt = pp.tile([128, 2, 2, 512], f32, tag="pt")

            for seg in range(2):
                t0, t1 = seg * SEG, min((seg + 1) * SEG, F)
                n = (t1 - t0) * B
                for mc in range(2):
                    nc.tensor.matmul(
                        pt[:, seg, mc, 0:n],
                        cfs[0:2, mc, :],
                        xbs[0:2, t0:t1, :], start=True, stop=True)

            def pslot(t):
                seg, off = t // SEG, B * (t % SEG)
                return pt[:, seg, :, off:off + B], pt[:, seg, 0, off:off + B], \
                    pt[:, seg, 1, off:off + B]

            h = None
            for t in range(F):
                both, p0, p1 = pslot(t)
                if t > 0:
                    for mc, px in ((0, p0), (1, p1)):
                        nc.tensor.matmul(px, whs[:, 0, mc, :], h[:, 0, :],
                                         start=False, stop=False,
                                         skip_group_check=True)
                        nc.tensor.matmul(px, whs[:, 1, mc, :], h[:, 1, :],
                                         start=False, stop=True,
                                         skip_group_check=True)
                ht = hp.tile([128, 2, B], f16, tag="h")
                nc.scalar.activation(ht[:], both, Tanh)
                h = ht

            nc.sync.dma_start(ph[:], h[:])

    nc.compile()
    return nc


# ---------------- host-side prep + glue ----------------
def _prep_conv_inputs(x, W1, b1):
    # im2col for conv1: stride==kernel => non-overlapping patches.
    # k-order (c, kh, kw); n-order (oh10, ow10, ph, pw) so the last free
    # axis of the psum groups each 2x2 maxpool window.
    xv = x.reshape(NF, C, 10, 2, 9, 10, 2, 9)
    pat = xv.transpose(0, 1, 4, 7, 2, 5, 3, 6).reshape(NF, KC1, N1)
    pat = pat.astype(np.float16)
    patp = np.zeros((NF, 256, N1), np.float16)
    patp[:, 0:KC1] = pat
    patp = patp.reshape(NCORES, NGRP, GRP, 2, 128, N1)
    pA = np.ascontiguousarray(patp[:, :, :, 0].transpose(0, 1, 3, 2, 4))
    pB = np.ascontiguousarray(patp[:, :, :, 1].transpose(0, 1, 3, 2, 4))

    w1m = np.zeros((2 * 128, 64), np.float16)
    w1m[:KC1] = W1.reshape(64, KC1).T               # [K, M]
    w1c = w1m.reshape(2, 128, 64).transpose(1, 0, 2)
    w1t = np.ascontiguousarray(np.concatenate([w1c, w1c], axis=2))

    b1d = np.concatenate([b1, b1]).reshape(128, 1).astype(np.float32)
    return pA, pB, w1t, b1d


def _host_conv2_bn(po_list, W2, b2, gamma, beta):
    # reassemble pool1 [NF, 64, 10, 10] from per-core [128, NPAIR, 100]
    pool1 = np.empty((NF, 64, 100), np.float32)
    for k, r in enumerate(po_list):
        v = np.asarray(r, np.float32)               # [128, 40, 100]
        base = k * FPC
        idx = base + 2 * np.arange(NPAIR)
        pool1[idx] = v[0:64].transpose(1, 0, 2)
        pool1[idx + 1] = v[64:128].transpose(1, 0, 2)
    pool1 = pool1.reshape(NF, 64, 10, 10)

    # conv2 5x5 stride 5 + relu + 2x2 maxpool -> y [B, F, 3]
    w2m = W2.reshape(3, 64 * 25)
    yv = pool1.reshape(NF, 64, 2, 5, 2, 5).transpose(0, 1, 3, 5, 2, 4)
    yv = np.ascontiguousarray(yv).reshape(NF, 64 * 25, 4)
    z = np.einsum('fkn,mk->fmn', yv, w2m, optimize=True) \
        + b2[None, :, None]
    z = np.maximum(z, 0.0).max(axis=2)              # [NF, 3]
    y = z.reshape(B, F, 3)

    mean = y.mean(axis=(0, 2), keepdims=True)
    var = y.var(axis=(0, 2), keepdims=True)
    yn = (y - mean) / np.sqrt(var + EPS) * gamma[None, :, None] \
        + beta[None, :, None]
    return yn


def _prep_rnn_blob(ts_r, Wih_r, Whh_r, bih_r, bhh_r, Wl):
    # ts_r: [F, B] f32 rank-r input sequence
    NX = F * B
    blob = np.zeros((128, NX + 512 + 256 + 10), np.float16)
    xb = np.zeros((128, F, B), np.float16)
    xb[0] = ts_r
    xb[1] = 1.0
    cf = np.zeros((128, 2, 128), np.float16)
    bsum = bih_r + bhh_r
    for mc in range(2):
        cf[0, mc, :] = Wih_r[mc * 128:(mc + 1) * 128, 0]
        cf[1, mc, :] = bsum[mc * 128:(mc + 1) * 128]
    blob[:, 0:256] = cf.reshape(128, 256)
    blob[:, 256:256 + NX] = xb.reshape(128, NX)
    wh = np.zeros((128, 2, 2, 128), np.float16)
    WhhT = Whh_r.T
    for kc in range(2):
        for mc in range(2):
            wh[:, kc, mc, :] = WhhT[kc * 128:(kc + 1) * 128,
                                    mc * 128:(mc + 1) * 128]
    blob[:, 256 + NX:256 + NX + 512] = wh.reshape(128, 512)
    wl = np.zeros((128, 2, 5), np.float16)
    WlT3 = (Wl.T / 3.0)
    for kc in range(2):
        wl[:, kc, :] = WlT3[kc * 128:(kc + 1) * 128]
    blob[:, 256 + NX + 512:] = wl.reshape(128, 10)
    return blob


def _ensure_profile_hook():
    """antenv.axon_hooks is absent in this image; synthesize it so
    run_bass_kernel_spmd(trace=True) can capture NTFF profiles."""
    import sys
    import types
    try:
        from antenv.axon_hooks import get_axon_ntff_profile_hook  # noqa
        return True
    except ImportError:
        pass
    try:
        sys.path.insert(0, "/root/.axon_site/trn_agent_boot")
        from trn_boot import _ntff_profile_via_ctypes
        hook = _ntff_profile_via_ctypes("/opt/axon/libaxon_pjrt.so")
        if hook is None:
            return False
        import antenv
        mod = types.ModuleType("antenv.axon_hooks")
        mod._hook = hook
        mod.get_axon_ntff_profile_hook = lambda: mod._hook
        mod.set_axon_ntff_profile_hook = lambda h: setattr(mod, "_hook", h)
        sys.modules["antenv.axon_hooks"] = mod
        antenv.axon_hooks = mod
        return True
    except Exception:
        return False


def _run(nc, in_maps, core_ids, label):
    from concourse.bass_utils import run_bass_kernel_spmd
    trace = os.environ.get("KERNEL_TRACE", "0") == "1"
    if trace:
        trace = _ensure_profile_hook()
    kw = {}
    if trace:
        import tempfile
        tdir = tempfile.mkdtemp(prefix=f"ktrace_{label}_")
        kw = {"tmpdir": tdir}
    res = run_bass_kernel_spmd(nc, in_maps, core_ids, trace=trace, **kw)
    _cache.setdefault("exec_ns", {})[label] = res.exec_time_ns
    _cache.setdefault("results_obj", {})[label] = res
    return res.results


# ---------------- main entry ----------------
def kernel(x, W1, b1, W2, b2, gamma, beta, Wih, Whh, bih, bhh, Wl, bl):
    x, W1, b1, W2, b2 = map(np.asarray, (x, W1, b1, W2, b2))
    gamma, beta = np.asarray(gamma), np.asarray(beta)
    Wih, Whh, bih, bhh = map(np.asarray, (Wih, Whh, bih, bhh))
    Wl, bl = np.asarray(Wl), np.asarray(bl)

    if "conv" not in _cache:
        _cache["conv"] = _build_conv_nc()
    if "rnn" not in _cache:
        _cache["rnn"] = _build_rnn_nc()

    # ---- launch A: conv1 + pool1 over 640 frames on 8 cores ----
    pA, pB, w1t, b1c = _prep_conv_inputs(x.reshape(NF, C, H, W), W1, b1)
    in_maps = [
        {"pA": pA[k], "pB": pB[k], "w1": w1t, "b1": b1c}
        for k in range(NCORES)
    ]
    res = _run(_cache["conv"], in_maps, list(range(NCORES)), "conv")

    # ---- host glue: conv2 + pool2 + BN + per-sample channel reorder ----
    yn = _host_conv2_bn([r["po"] for r in res], W2, b2, gamma, beta)
    t = yn.transpose(0, 2, 1)                        # [B, 3, F]
    rng = t.max(-1) - t.min(-1)
    perm = np.argsort(rng, axis=1, kind="stable")
    tsel = np.take_along_axis(t, perm[:, :, None], axis=1)  # [B, 3, F]

    # ---- launch B: 3 RNNs on 3 cores (+ scaled final linear) ----
    in_maps_b = []
    for r in range(3):
        blob = _prep_rnn_blob(tsel[:, r, :].T, Wih[r], Whh[r],
                              bih[r], bhh[r], Wl)
        in_maps_b.append({"blob": blob})
    res_b = _run(_cache["rnn"], in_maps_b, [0, 1, 2], "rnn")

    avg = np.zeros((B, OUT), np.float32)
    for r in range(3):
        hv = np.asarray(res_b[r]["ph"], np.float32)      # [128, 2, B]
        hr = hv.transpose(1, 0, 2).reshape(OUT, B)       # [256, B]
        avg += hr.T / 3.0
    out = avg @ Wl.T + bl[None, :]
    return out.astype(np.float32)
